# revision 18
# baseline (speedup 1.0000x reference)
"""Cross-attention kernel for Trainium2, data-parallel over batch on 8 NeuronCores.

Per core (batch element b):
  q = x[b] @ Wq.T + bq ; k = c[b] @ Wk.T + bk ; v = c[b] @ Wv.T + bv
  out[b] = softmax(q @ k.T / sqrt(D)) @ v

Device layout (all matmul operands bf16, fp32 accumulation):
  phase 1: QT[e,s] = (Wq @ x.T + bq)/sqrt(D), KT[e,t] = Wk @ c.T + bk   (e on partitions)
           V[t,e]  = c @ Wv.T + bv                                      (t on partitions)
  phase 2: per 128-row tile of s: S = QT.T @ KT (psum), P = exp(S) + row sums via ACT,
           P transposed 128x128 via DMA xbar, O = P @ V, scaled by 1/rowsum on drain.
"""

import numpy as np
import ml_dtypes

import concourse.bass as bass
import concourse.mybir as mybir
import concourse.tile as tile
from concourse import bacc
from concourse.bass_utils import run_bass_kernel_spmd

DIM = 1024
SEQ = 2048
B = 8
P = 128
DT = DIM // P        # 8 contraction tiles of 128
ST = SEQ // P        # 16 seq tiles of 128
KC = SEQ // 512      # 4 key chunks of 512
EC = DIM // 512      # 2 embed chunks of 512
F32 = mybir.dt.float32
BF16 = mybir.dt.bfloat16

_CACHED_NC = None


def build_nc():
    nc = bacc.Bacc(None, target_bir_lowering=False)

    xt = nc.declare_dram_parameter("xt", [DIM, SEQ], BF16, isOutput=False)
    ct = nc.declare_dram_parameter("ct", [DIM, SEQ], BF16, isOutput=False)
    wqt = nc.declare_dram_parameter("wqt", [DIM, DIM], BF16, isOutput=False)
    wkt = nc.declare_dram_parameter("wkt", [DIM, DIM], BF16, isOutput=False)
    wvt = nc.declare_dram_parameter("wvt", [DIM, DIM], BF16, isOutput=False)
    bqs = nc.declare_dram_parameter("bqs", [DT, P], F32, isOutput=False)
    bks = nc.declare_dram_parameter("bks", [DT, P], F32, isOutput=False)
    bvb = nc.declare_dram_parameter("bvb", [P, DIM], F32, isOutput=False)
    out = nc.declare_dram_parameter("out", [SEQ, DIM], F32, isOutput=True)

    xt_r = xt.rearrange("(t p) s -> p t s", p=P)
    ct_r = ct.rearrange("(t p) s -> p t s", p=P)
    wqt_r = wqt.rearrange("(t p) e -> p t e", p=P)
    wkt_r = wkt.rearrange("(t p) e -> p t e", p=P)
    wvt_r = wvt.rearrange("(t p) e -> p t e", p=P)
    out_r = out.rearrange("(t p) e -> p t e", p=P)

    with tile.TileContext(nc) as tc:
        with (
            tc.tile_pool(name="resid", bufs=1) as resid,
            tc.tile_pool(name="singles", bufs=1) as singles,
        ):
            qt_sb = resid.tile([P, DT, SEQ], BF16, tag="qt")
            kt_sb = resid.tile([P, DT, SEQ], BF16, tag="kt")
            v_sb = resid.tile([P, ST, DIM], BF16, tag="v")

            bq_sb = singles.tile([P, DT], F32, tag="bq")
            bk_sb = singles.tile([P, DT], F32, tag="bk")
            bv_sb = singles.tile([P, DIM], F32, tag="bv")

            # ---------------- phase 1: projections ----------------
            # xt (feeds q, which runs first) loads on the HWDGE queue while
            # ct / wv load on the SWDGE queue, so the first q matmul is not
            # stuck behind 14 MB of serialized input DMA.
            with (
                tc.tile_pool(name="acts", bufs=1) as acts,
                tc.tile_pool(name="wpool", bufs=3) as wpool,
                tc.tile_pool(name="wvpool", bufs=1) as wvpool,
                tc.tile_pool(name="warmps", bufs=1, space="PSUM") as warmps,
                tc.tile_pool(name="ppool", bufs=6, space="PSUM") as ppool,
            ):
                # Dummy matmuls on a zeroed tile keep the PE busy through the
                # input-DMA window: HAM un-throttles to 2.4 GHz before the
                # real matmuls start, instead of ramping on them.
                wsrc = acts.tile([P, 512], BF16, tag="warm")
                nc.vector.memset(wsrc, 0.0)
                wps = warmps.tile([P, 512], F32, tag="wps")
                for i in range(20):
                    nc.tensor.matmul(
                        wps, wsrc[:, 0:P], wsrc, start=(i == 0), stop=(i == 19)
                    )
                xt_sb = acts.tile([P, DT, SEQ], BF16, tag="xt")
                ct_sb = acts.tile([P, DT, SEQ], BF16, tag="ct")
                wv_t = wvpool.tile([P, DT, DIM], BF16, tag="wv")
                # Biases first (tiny, needed by the first PSUM drain), then
                # xt split across all three DMA queues so the first q matmul
                # starts as early as possible; ct/wv follow on gpsimd.
                nc.gpsimd.dma_start(out=bq_sb, in_=bqs.rearrange("t p -> p t"))
                nc.gpsimd.dma_start(out=bk_sb, in_=bks.rearrange("t p -> p t"))
                for dt in range(3):
                    nc.sync.dma_start(out=xt_sb[:, dt], in_=xt_r[:, dt])
                for dt in range(3, 6):
                    nc.scalar.dma_start(out=xt_sb[:, dt], in_=xt_r[:, dt])
                for dt in range(6, 8):
                    nc.gpsimd.dma_start(out=xt_sb[:, dt], in_=xt_r[:, dt])
                w_t0 = wpool.tile([P, DT, P], BF16, tag="w")
                nc.gpsimd.dma_start(out=w_t0, in_=wqt_r[:, :, 0:P])
                nc.gpsimd.dma_start(out=bv_sb, in_=bvb[:, :])
                nc.gpsimd.dma_start(out=ct_sb[:, :], in_=ct_r[:, :, :])
                nc.gpsimd.dma_start(out=wv_t, in_=wvt_r)

                # q and k projections: out[e128, s512] accumulated over d
                for (w_r, src_sb, dst_sb, b_sb, scale) in (
                    (wqt_r, xt_sb, qt_sb, bq_sb, 1.0 / 32.0),
                    (wkt_r, ct_sb, kt_sb, bk_sb, 1.0),
                ):
                    for et in range(DT):
                        if w_r is wqt_r and et == 0:
                            w_t = w_t0
                        else:
                            w_t = wpool.tile([P, DT, P], BF16, tag="w")
                            # q weights prefetch on the scalar HWDGE queue
                            # (behind xt dt3-5 only); k weights on sync.
                            eng = nc.scalar if w_r is wqt_r else nc.sync
                            eng.dma_start(
                                out=w_t, in_=w_r[:, :, et * P : (et + 1) * P]
                            )
                        for sc in range(KC):
                            ps = ppool.tile([P, 512], F32, tag="proj")
                            for dt in range(DT):
                                nc.tensor.matmul(
                                    ps,
                                    w_t[:, dt],
                                    src_sb[:, dt, sc * 512 : (sc + 1) * 512],
                                    start=(dt == 0),
                                    stop=(dt == DT - 1),
                                )
                            nc.scalar.activation(
                                out=dst_sb[:, et, sc * 512 : (sc + 1) * 512],
                                in_=ps,
                                func=mybir.ActivationFunctionType.Identity,
                                bias=b_sb[:, et : et + 1],
                                scale=scale,
                            )

                # v projection: out[t128, e512], CT tiles stationary
                for tt in range(ST):
                    for ec in range(EC):
                        ps = ppool.tile([P, 512], F32, tag="proj")
                        for dt in range(DT):
                            nc.tensor.matmul(
                                ps,
                                ct_sb[:, dt, tt * P : (tt + 1) * P],
                                wv_t[:, dt, ec * 512 : (ec + 1) * 512],
                                start=(dt == 0),
                                stop=(dt == DT - 1),
                            )
                        nc.vector.tensor_add(
                            out=v_sb[:, tt, ec * 512 : (ec + 1) * 512],
                            in0=ps,
                            in1=bv_sb[:, ec * 512 : (ec + 1) * 512],
                        )

            # ---------------- phase 2: attention ----------------
            # Software-pipelined: S/exp/transpose for tile st is emitted
            # before O/store for tile st-1, so the PE chews on S(st) while
            # the xbar transpose of P(st-1) completes.
            with (
                tc.tile_pool(name="attn", bufs=3) as attn,
                tc.tile_pool(name="stats", bufs=4) as stats,
                tc.tile_pool(name="spsum", bufs=5, space="PSUM") as spsum,
                tc.tile_pool(name="opsum", bufs=3, space="PSUM") as opsum,
            ):
                def emit_s_stage(st):
                    p_sb = attn.tile([P, SEQ], BF16, tag="p")
                    sums = stats.tile([P, KC], F32, tag="sums")
                    for kc in range(KC):
                        sp = spsum.tile([P, 512], F32, tag="s")
                        for dt in range(DT):
                            nc.tensor.matmul(
                                sp,
                                qt_sb[:, dt, st * P : (st + 1) * P],
                                kt_sb[:, dt, kc * 512 : (kc + 1) * 512],
                                start=(dt == 0),
                                stop=(dt == DT - 1),
                            )
                        nc.scalar.activation(
                            out=p_sb[:, kc * 512 : (kc + 1) * 512],
                            in_=sp,
                            func=mybir.ActivationFunctionType.Exp,
                            accum_out=sums[:, kc : kc + 1],
                        )
                    ssum = stats.tile([P, 1], F32, tag="ssum")
                    rsum = stats.tile([P, 1], F32, tag="rsum")
                    nc.vector.reduce_sum(out=ssum, in_=sums, axis=mybir.AxisListType.X)
                    nc.vector.reciprocal(out=rsum, in_=ssum)

                    # One xbar transpose for the whole row block:
                    # pt[p, tt, f] = p_sb[f, tt*128 + p]
                    pt_sb = attn.tile([P, ST, P], BF16, tag="pt")
                    nc.sync.dma_start_transpose(out=pt_sb, in_=p_sb[:, :])
                    return pt_sb, rsum

                def emit_o_stage(st, pt_sb, rsum):
                    o_sb = attn.tile([P, DIM], F32, tag="o")
                    for ec in range(EC):
                        op = opsum.tile([P, 512], F32, tag="o")
                        for tt in range(ST):
                            nc.tensor.matmul(
                                op,
                                pt_sb[:, tt],
                                v_sb[:, tt, ec * 512 : (ec + 1) * 512],
                                start=(tt == 0),
                                stop=(tt == ST - 1),
                            )
                        nc.vector.tensor_scalar_mul(
                            out=o_sb[:, ec * 512 : (ec + 1) * 512],
                            in0=op,
                            scalar1=rsum,
                        )
                    nc.gpsimd.dma_start(out=out_r[:, st], in_=o_sb)

                pending = None
                for st in range(ST):
                    cur = emit_s_stage(st)
                    if pending is not None:
                        emit_o_stage(st - 1, *pending)
                    pending = cur
                emit_o_stage(ST - 1, *pending)

    nc.compile()
    return nc


def prep_inputs(x, context, Wq, bq, Wk, bk, Wv, bv):
    """Host-side prep: per-batch transposed bf16 activations, transposed bf16
    weights, tiled fp32 biases. Returns per-core input maps."""
    bf = ml_dtypes.bfloat16
    wqt = np.ascontiguousarray(Wq.T).astype(bf)
    wkt = np.ascontiguousarray(Wk.T).astype(bf)
    wvt = np.ascontiguousarray(Wv.T).astype(bf)
    bqs = (bq.astype(np.float32) / 32.0).reshape(DT, P)
    bks = bk.astype(np.float32).reshape(DT, P)
    bvb = np.ascontiguousarray(
        np.broadcast_to(bv.astype(np.float32), (P, DIM))
    )
    in_maps = []
    for b in range(B):
        in_maps.append(
            {
                "xt": np.ascontiguousarray(x[b].T).astype(bf),
                "ct": np.ascontiguousarray(context[b].T).astype(bf),
                "wqt": wqt,
                "wkt": wkt,
                "wvt": wvt,
                "bqs": bqs,
                "bks": bks,
                "bvb": bvb,
            }
        )
    return in_maps


def kernel(x, context, Wq, bq, Wk, bk, Wv, bv):
    global _CACHED_NC
    x = np.asarray(x, dtype=np.float32)
    context = np.asarray(context, dtype=np.float32)
    in_maps = prep_inputs(x, context, np.asarray(Wq), np.asarray(bq),
                          np.asarray(Wk), np.asarray(bk),
                          np.asarray(Wv), np.asarray(bv))
    if _CACHED_NC is None:
        _CACHED_NC = build_nc()
    nc = _CACHED_NC
    core_ids = list(range(B))
    res = run_bass_kernel_spmd(nc, in_maps, core_ids)
    return np.stack([res.results[i]["out"] for i in core_ids]).astype(np.float32)
